# revision 38
# baseline (speedup 1.0000x reference)
"""Tensor-parallel GQA multi-head attention (RoPE + causal softmax) for 8 trn2 cores.

Sharding v2: every core handles BOTH batches with 4 q-heads / 1 kv-head:
core c owns q-heads {4c..4c+3} (kv-head c) of batches 0 and 1. Attention
runs in transposed (feature-major) layout with flash-style causal tiling.
Per 512-token slab, the 8 cores exchange their normalized attention outputs
with one AllToAll (bf16, 512KB) so that core c ends up with ALL 2048
attention features for its 128-position output stripe (batch c//4, stripe
c%4); it then applies the full wo to produce disjoint output rows. No
reduction collective is needed.
"""

import sys

sys.path.insert(0, "/opt/trn_rl_repo")

import numpy as np

import concourse.bass as bass
import concourse.bacc as bacc
import concourse.mybir as mybir
from concourse import tile
from concourse.bass_utils import run_bass_kernel_spmd

B, S, D = 2, 2048, 2048
N_HEADS, N_KV, HD = 32, 8, 64
NCORES = 8
QH = 4    # q-heads per core
FQ = QH * HD       # 256 q-feature cols per core
FKV = 2 * HD       # 128 (K then V) per core
SCALE = 1.0 / 8.0  # 1/sqrt(HD)

QTILE = 512
KTILE = 128
NSLAB = S // QTILE  # 4
ND = D // 128       # 16 contraction chunks

F32 = mybir.dt.float32
EXP = mybir.ActivationFunctionType.Exp
BF16 = mybir.dt.bfloat16
MMD = BF16
LE = mybir.AluOpType.is_ge


def _build_kernel(tc, io):
    nc = tc.nc
    xT, wq, wkv, wo = io["xT"], io["wq"], io["wkv"], io["wo"]
    cos2, sin2s, sel = io["cos2"], io["sin2s"], io["sel"]
    out_full = io["out"]
    single = bool(io.get("single"))

    # ---------------- pools ----------------
    const = tc.alloc_tile_pool(name="const", bufs=1)
    wpool = tc.alloc_tile_pool(name="wpool", bufs=1, side="right")
    kvp = tc.alloc_tile_pool(name="kvp", bufs=1)
    xpool = tc.alloc_tile_pool(name="xpool", bufs=2)
    qpool = tc.alloc_tile_pool(name="qpool", bufs=2)
    aop = tc.alloc_tile_pool(name="aop", bufs=2, side="right")
    rp = tc.alloc_tile_pool(name="rp", bufs=2)
    pexp = tc.alloc_tile_pool(name="pexp", bufs=4)
    evac = tc.alloc_tile_pool(name="evac", bufs=2)
    aogp = tc.alloc_tile_pool(name="aogp", bufs=2, side="right")
    dram = tc.alloc_tile_pool(name="dram", bufs=1, space="DRAM")

    psM = tc.alloc_tile_pool(name="psM", bufs=2, space="PSUM")
    psS = tc.alloc_tile_pool(name="psS", bufs=2, space="PSUM")
    psO = tc.alloc_tile_pool(name="psO", bufs=1, space="PSUM")

    # ------- constants + weights; DMA order tuned for fast start -------
    cos2_t = const.tile([128, S], MMD)
    nc.sync.dma_start(cos2_t[:], cos2[:])
    sin2s_t = const.tile([128, S], MMD)
    nc.sync.dma_start(sin2s_t[:], sin2s[:])
    ident = const.tile([128, 64], F32)
    nc.gpsimd.memset(ident[:], 0.0)
    for p in (0, 64):
        nc.gpsimd.affine_select(
            out=ident[p:p + 64, :], in_=ident[p:p + 64, :],
            compare_op=mybir.AluOpType.not_equal,
            fill=1.0, base=0, pattern=[[-1, 64]], channel_multiplier=1,
        )
    # x slab for (b=0, j=0) interleaved with the projection weights, spread
    # over both HW DMA queues so the first matmul group starts within ~5us
    xts00 = []
    WQ = {}
    WKV = {}
    for k in range(ND):
        q1, q2 = (nc.sync, nc.scalar) if k % 2 == 0 else (nc.scalar, nc.sync)
        xt = xpool.tile([128, QTILE], MMD, name="xt", tag=f"xt{k}")
        q1.dma_start(xt[:], xT[k * 128:(k + 1) * 128, 0:QTILE])
        xts00.append(xt)
        for t in range(2):
            w = wpool.tile([128, 128], MMD, name=f"wq{t}_{k}")
            q2.dma_start(w[:], wq[k * 128:(k + 1) * 128,
                                  t * 128:(t + 1) * 128])
            WQ[t, k] = w
        w = wpool.tile([128, 128], MMD, name=f"wkv{k}")
        q1.dma_start(w[:], wkv[k * 128:(k + 1) * 128, :])
        WKV[k] = w

    sel_t = const.tile([2 * QH, 4 * KTILE], MMD)
    nc.sync.dma_start(sel_t[:], sel[:])

    # full wo (loaded via the scalar DMA queue; scalar is idle early on)
    WO = {}
    for fc in range(ND):
        for dn in range(4):
            w = wpool.tile([128, QTILE], MMD, name=f"wo{fc}_{dn}")
            nc.scalar.dma_start(
                w[:], wo[fc * 128:(fc + 1) * 128,
                         dn * QTILE:(dn + 1) * QTILE])
            WO[fc, dn] = w

    # persistent K/V cache tiles
    KK = [kvp.tile([128, S], MMD, name=f"kk{b}") for b in range(B)]
    VA = {}
    for b in range(B):
        for i in range(S // KTILE):
            VA[b, i] = kvp.tile([128, HD + 1], MMD, name=f"va{b}_{i}")

    # A2A dram tiles (one pair per slab)
    a2a_in = [dram.tile([FQ * NCORES, KTILE], MMD, name=f"ain{j}")
              for j in range(NSLAB)]
    a2a_out = [dram.tile([FQ * NCORES, KTILE], MMD, name=f"aout{j}")
               for j in range(NSLAB)]

    AO = {}   # per (b, t) slab-local attention output, feature-major
    QT = {}

    def rope(dst, rows, qs, tab_qs):
        # dst[rows, qs] = dst*cos + swap32(dst)*sin  (feature-major RoPE);
        # qs indexes dst columns, tab_qs the (global-position) rope tables
        n = rows[1] - rows[0]
        qsw = rp.tile([128, QTILE], MMD, name="qsw", tag="qsw")
        for p in range(rows[0], rows[1], 64):
            q0 = p - rows[0]
            nc.sync.dma_start(qsw[q0:q0 + 32, :], dst[p + 32:p + 64, qs])
            nc.sync.dma_start(qsw[q0 + 32:q0 + 64, :], dst[p:p + 32, qs])
        t1 = rp.tile([128, QTILE], F32, name="t1", tag="t1")
        nc.vector.tensor_mul(t1[:n], dst[rows[0]:rows[1], qs],
                             cos2_t[rows[0]:rows[1], tab_qs])
        t2 = rp.tile([128, QTILE], F32, name="t2", tag="t2")
        nc.vector.tensor_mul(t2[:n], qsw[:n], sin2s_t[rows[0]:rows[1], tab_qs])
        nc.vector.tensor_add(dst[rows[0]:rows[1], qs], t1[:n], t2[:n])

    def prefetch_x(b, j):
        qs = slice(j * QTILE, (j + 1) * QTILE)
        xts = []
        for k in range(ND):
            xt = xpool.tile([128, QTILE], MMD, name="xt", tag=f"xt{k}")
            nc.sync.dma_start(
                xt[:], xT[b * D + k * 128:b * D + (k + 1) * 128, qs])
            xts.append(xt)
        return xts

    def make_proj_fillers(b, j, xts):
        # projection for (b, j), split into small PE chunks so it can be
        # woven into the preceding attention's exp-bound inner loop
        qs = slice(j * QTILE, (j + 1) * QTILE)
        ctx = {}
        fillers = []
        if xts is not None:
            ctx["x"] = xts
        else:
            def loadx():
                ctx["x"] = prefetch_x(b, j)
            fillers.append(loadx)
        for f in range(3):
            for sub in range(4):
                def mmchunk(f=f, sub=sub):
                    if sub == 0:
                        ctx[f] = psM.tile([128, QTILE], F32, name="psq",
                                          tag="mm")
                    ps = ctx[f]
                    for k in range(4 * sub, 4 * sub + 4):
                        w = WQ[f, k] if f < 2 else WKV[k]
                        nc.tensor.matmul(ps[:], w[:], ctx["x"][k][:],
                                         start=(k == 0), stop=(k == ND - 1))
                fillers.append(mmchunk)

            def evacf(f=f):
                ps = ctx[f]
                if f < 2:
                    qt = qpool.tile([128, QTILE], MMD, name="qt",
                                    tag=f"qt{b}_{f}")
                    QT[b, f] = qt
                    nc.vector.tensor_copy(qt[:], ps[:])
                    rope(qt, (0, 128), slice(0, QTILE), qs)
                else:
                    nc.vector.tensor_copy(KK[b][0:64, qs], ps[0:64, :])
                    rope(KK[b], (0, 64), qs, qs)
                    # duplicate roped K into rows 64:128 (row-tiled scores)
                    nc.sync.dma_start(KK[b][64:128, qs], KK[b][0:64, qs])
                    vv = rp.tile([128, QTILE], F32, name="vv", tag="vv")
                    nc.vector.tensor_copy(vv[64:128, :], ps[64:128, :])
                    ctx["vv"] = vv
            fillers.append(evacf)
        for c in range(4):
            def vtrans(c=c):
                i = 4 * j + c
                tp = psM.tile([128, QTILE], F32, name="tp", tag="mm")
                vv = ctx["vv"]
                nc.tensor.matmul(tp[:, 0:HD],
                                 vv[64:128, c * 128:(c + 1) * 128],
                                 ident[64:128, :], is_transpose=True,
                                 start=True, stop=True)
                va = VA[b, i]
                nc.vector.tensor_copy(va[:, 0:HD], tp[:, 0:HD])
                nc.vector.memset(va[:, HD:HD + 1], 1.0)
            fillers.append(vtrans)
        return fillers

    def proj(b, j, xts=None):
        for f in make_proj_fillers(b, j, xts):
            f()

    def attn(b, j, fillers=None):
        # fillers: closures emitting small independent PE chunks (wo/proj
        # work); paced evenly through the loop and placed before each attnV
        # so the tensor engine has work while it would otherwise stall on
        # the exp
        fillers = list(fillers or [])
        nkt = 4 * j + 4
        slots = 2 * nkt
        rate = len(fillers) / slots if slots else 0.0
        acc = [0.0]

        def pop_fillers():
            acc[0] += rate
            while fillers and acc[0] >= 1.0:
                fillers.pop(0)()
                acc[0] -= 1.0
        for t in range(2):
            oA = psO.tile([HD + 1, QTILE], F32, name="oA", tag="oA")
            oB = psO.tile([HD + 1, QTILE], F32, name="oB", tag="oB")
            sabs = {}

            def scores(i):
                r = i - 4 * j
                off = max(r, 0) * KTILE
                ks = slice(i * KTILE, (i + 1) * KTILE)
                sAB = psS.tile([128, 2 * QTILE], F32, name="sAB", tag="sAB")
                nc.tensor.matmul(sAB[:, off:QTILE], KK[b][0:64, ks],
                                 QT[b, t][0:64, off:], start=True, stop=True,
                                 tile_position=(0, 0))
                nc.tensor.matmul(sAB[:, QTILE + off:], KK[b][64:128, ks],
                                 QT[b, t][64:128, off:], start=True, stop=True,
                                 tile_position=(64, 0))
                sabs[i] = sAB

            scores(0)
            for i in range(nkt):
                r = i - 4 * j
                off = max(r, 0) * KTILE
                if i + 1 < nkt:
                    scores(i + 1)
                sAB = sabs.pop(i)
                pAB = pexp.tile([128, 2 * QTILE], MMD, name="pAB", tag="pAB")
                nc.scalar.activation(pAB[:, off:], sAB[:, off:], EXP,
                                     scale=SCALE)
                if r >= 0:
                    for h in (off, QTILE + off):
                        # zero the strictly-upper triangle (causal mask);
                        # gpsimd: keeps the DVE queue off this critical path
                        nc.gpsimd.affine_select(
                            out=pAB[:, h:h + KTILE], in_=pAB[:, h:h + KTILE],
                            compare_op=LE, fill=0.0, base=0,
                            pattern=[[1, KTILE]], channel_multiplier=-1)
                pop_fillers()
                nc.tensor.matmul(oA[:, off:], VA[b, i][:], pAB[:, off:QTILE],
                                 start=(i == 0), stop=(i == nkt - 1))
                nc.tensor.matmul(oB[:, off:], VA[b, i][:], pAB[:, QTILE + off:],
                                 start=(i == 0), stop=(i == nkt - 1))
            tA = evac.tile([HD + 1, QTILE], MMD, name="tA", tag="tA")
            tB = evac.tile([HD + 1, QTILE], MMD, name="tB", tag="tB")
            nc.vector.tensor_copy(tA[:], oA[:])
            nc.vector.tensor_copy(tB[:], oB[:])
            ao = AO[b, t]
            nc.sync.dma_start(ao[0:64, :], tA[0:64, :])
            nc.sync.dma_start(ao[64:128, :], tB[0:64, :])
            dn = AO["dn"]
            nc.sync.dma_start(dn[4 * b + t:4 * b + t + 1, :], tA[64:65, :])
            nc.sync.dma_start(dn[4 * b + 2 + t:4 * b + 3 + t, :],
                              tB[64:65, :])
        for f in fillers:
            f()

    def finish(j, AOj):
        # normalize, build the A2A input, kick the A2A
        dn = AOj["dn"]
        dnR = evac.tile([2 * QH, QTILE], F32, name="dnR", tag="dnR")
        nc.vector.reciprocal(dnR[:], dn[:])
        dnRb = evac.tile([2 * QH, QTILE], MMD, name="dnRb", tag="dnRb")
        nc.vector.tensor_copy(dnRb[:], dnR[:])
        for b in range(B):
            for t in range(2):
                bc = psM.tile([128, QTILE], F32, name="bc", tag="mm")
                nc.tensor.matmul(
                    bc[:], sel_t[:, (2 * b + t) * 128:(2 * b + t + 1) * 128],
                    dnRb[:], start=True, stop=True)
                nc.vector.tensor_mul(AOj[b, t][:], AOj[b, t][:], bc[:])
        for d in range(NCORES):
            bd, g = d // 4, d % 4
            for t in range(2):
                nc.sync.dma_start(
                    a2a_in[j][FQ * d + 128 * t:FQ * d + 128 * (t + 1), :],
                    AOj[bd, t][:, g * KTILE:(g + 1) * KTILE])
        if single:
            nc.sync.dma_start(a2a_out[j][:], a2a_in[j][:])
        else:
            nc.gpsimd.collective_compute(
                "AllToAll", mybir.AluOpType.bypass,
                replica_groups=[list(range(NCORES))],
                ins=[a2a_in[j][:]], outs=[a2a_out[j][:]],
            )

    def make_wo_fillers(j):
        # wo for slab j, split into one gather step plus 4x4 matmul chunks
        ctx = {}

        def gather():
            ctx["aogs"] = []
            for fc in range(ND):
                aog = aogp.tile([128, KTILE], MMD, name="aog", tag=f"aog{fc}")
                nc.sync.dma_start(aog[:],
                                  a2a_out[j][fc * 128:(fc + 1) * 128, :])
                ctx["aogs"].append(aog)

        fillers = [gather]
        for dn in range(4):
            for sub in range(4):
                def chunk(dn=dn, sub=sub):
                    if sub == 0:
                        ctx[dn] = psM.tile([128, QTILE], F32, name="psW",
                                           tag="mm")
                    ps = ctx[dn]
                    for fc in range(4 * sub, 4 * sub + 4):
                        nc.tensor.matmul(ps[:], ctx["aogs"][fc][:],
                                         WO[fc, dn][:],
                                         start=(fc == 0), stop=(fc == ND - 1))
                    if sub == 3:
                        og = evac.tile([128, QTILE], F32, name="og", tag="og")
                        nc.vector.tensor_copy(og[:], ps[:])
                        nc.gpsimd.dma_start(
                            out_full[j * 128:(j + 1) * 128,
                                     dn * QTILE:(dn + 1) * QTILE], og[:])
                fillers.append(chunk)
        return fillers

    def wo_slab(j):
        for f in make_wo_fillers(j):
            f()

    def interleave(*lists):
        out = []
        idx = [0] * len(lists)
        while any(i < len(l) for i, l in zip(idx, lists)):
            for n, l in enumerate(lists):
                if idx[n] < len(l):
                    out.append(l[idx[n]])
                    idx[n] += 1
        return out

    proj(0, 0, xts00)
    pend = None
    for j in range(NSLAB):
        AO.clear()
        AO["dn"] = aop.tile([2 * QH, QTILE], MMD, name="dn", tag="dn")
        for b in range(B):
            AO[b, 0] = aop.tile([128, QTILE], MMD, name=f"ao{b}0",
                                tag=f"ao{b}0")
            AO[b, 1] = aop.tile([128, QTILE], MMD, name=f"ao{b}1",
                                tag=f"ao{b}1")
        if pend is not None:
            finish(*pend)
        # weave proj(1, j) into attn(0, j)'s exp-bound loop
        attn(0, j, fillers=make_proj_fillers(1, j, None if j == 0 else nxt1))
        # weave the previous slab's wo and the next slab's proj(0) into
        # attn(1, j)
        f_wo = make_wo_fillers(j - 1) if j > 0 else []
        if j + 1 < NSLAB:
            nxt0 = prefetch_x(0, j + 1)
            f_pj = make_proj_fillers(0, j + 1, nxt0)
        else:
            f_pj = []
        attn(1, j, fillers=interleave(f_wo, f_pj))
        nxt1 = prefetch_x(1, j + 1) if j + 1 < NSLAB else None
        pend = (j, dict(AO))
    finish(*pend)
    wo_slab(NSLAB - 1)

    for p in (psO, psS, psM, dram, aogp, evac, pexp, rp, aop, qpool, xpool,
              kvp, wpool, const):
        p.release()


def _build(single=False):
    nc = bacc.Bacc("TRN2", target_bir_lowering=False, debug=False,
                   num_devices=1 if single else NCORES)
    io = {
        "xT": nc.dram_tensor("xT", [B * D, S], BF16, kind="ExternalInput").ap(),
        "wq": nc.dram_tensor("wq", [D, FQ], BF16, kind="ExternalInput").ap(),
        "wkv": nc.dram_tensor("wkv", [D, FKV], BF16, kind="ExternalInput").ap(),
        "wo": nc.dram_tensor("wo", [D, D], BF16, kind="ExternalInput").ap(),
        "cos2": nc.dram_tensor("cos2", [128, S], BF16, kind="ExternalInput").ap(),
        "sin2s": nc.dram_tensor("sin2s", [128, S], BF16, kind="ExternalInput").ap(),
        "sel": nc.dram_tensor("sel", [2 * QH, 4 * KTILE], BF16,
                              kind="ExternalInput").ap(),
        "out": nc.dram_tensor("out", [NSLAB * 128, D], F32,
                              kind="ExternalOutput").ap(),
    }
    io["single"] = single
    with tile.TileContext(nc) as tc:
        _build_kernel(tc, io)
    nc.compile()
    return nc


_CACHE = {}


def _get_program():
    if "nc" not in _CACHE:
        _CACHE["nc"] = _build()
    return _CACHE["nc"]


def _host_inputs(x, wq, wk, wv, wo):
    x = np.ascontiguousarray(x, np.float32)
    inv = 1.0 / (10000.0 ** (np.arange(0, HD, 2, dtype=np.float64) / HD))
    pos = np.arange(S, dtype=np.float64)
    freqs = np.outer(pos, inv)                      # [S, 32]
    emb = np.concatenate([freqs, freqs], axis=1)    # [S, 64]
    cos = np.cos(emb).T.astype(np.float32)          # [64, S]
    sin = np.sin(emb).T.astype(np.float32)
    cos2 = np.concatenate([cos, cos], axis=0)       # [128, S]
    sin2s = np.concatenate([-sin[:32], sin[32:], -sin[:32], sin[32:]], axis=0)

    # denominator broadcast selector: for (b, t) block, AO[b,t] rows 0:64
    # <- dn row 4b+t, rows 64:128 <- dn row 4b+2+t
    sel = np.zeros((2 * QH, 4 * KTILE), np.float32)
    for b in range(2):
        for t in range(2):
            blk = (2 * b + t) * 128
            sel[4 * b + t, blk:blk + 64] = 1.0
            sel[4 * b + 2 + t, blk + 64:blk + 128] = 1.0

    import ml_dtypes
    bf16 = ml_dtypes.bfloat16
    cos2 = cos2.astype(bf16)
    sin2s = sin2s.astype(bf16)
    sel = sel.astype(bf16)
    xT = np.ascontiguousarray(
        np.concatenate([x[0].T, x[1].T], axis=0).astype(bf16))  # [2D, S]

    # wo rows ordered to match the gathered A2A feature order:
    # src core cc contributes heads (4cc+t, 4cc+t+2) for t in (0, 1)
    wrows = []
    for cc in range(NCORES):
        for t in range(2):
            for h in (4 * cc + t, 4 * cc + t + 2):
                wrows.append(wo[h * HD:(h + 1) * HD, :])
    wo_p = np.ascontiguousarray(np.concatenate(wrows, axis=0).astype(bf16))

    in_maps = []
    for c in range(NCORES):
        qcols = []
        for t in range(2):
            for h in (4 * c + t, 4 * c + t + 2):
                qcols.append(wq[:, h * HD:(h + 1) * HD])
        wq_p = np.ascontiguousarray(np.concatenate(qcols, axis=1).astype(bf16))
        wkv_p = np.ascontiguousarray(np.concatenate(
            [wk[:, c * HD:(c + 1) * HD], wv[:, c * HD:(c + 1) * HD]],
            axis=1).astype(bf16))
        in_maps.append({
            "xT": xT, "wq": wq_p, "wkv": wkv_p, "wo": wo_p,
            "cos2": cos2, "sin2s": sin2s, "sel": sel,
        })
    return in_maps


def run(x, wq, wk, wv, wo, trace=False, **trace_kwargs):
    nc = _get_program()
    in_maps = _host_inputs(x, wq, wk, wv, wo)
    res = run_bass_kernel_spmd(nc, in_maps, list(range(NCORES)),
                               trace=trace, **trace_kwargs)
    out = np.empty((B, S, D), np.float32)
    for c in range(NCORES):
        bo, g = c // 4, c % 4
        shard = res.results[c]["out"]  # [512, D]
        for j in range(NSLAB):
            out[bo, j * QTILE + g * 128:j * QTILE + (g + 1) * 128, :] = \
                shard[j * 128:(j + 1) * 128, :]
    return out, res


def kernel(x, wq, wk, wv, wo):
    out, _ = run(x, wq, wk, wv, wo)
    return out.astype(np.float32)


# revision 39
# speedup vs baseline: 1.1319x; 1.1319x over previous
"""Tensor-parallel GQA multi-head attention (RoPE + causal softmax) for 8 trn2 cores.

Sharding v2: every core handles BOTH batches with 4 q-heads / 1 kv-head:
core c owns q-heads {4c..4c+3} (kv-head c) of batches 0 and 1. Attention
runs in transposed (feature-major) layout with flash-style causal tiling.
Per 512-token slab, the 8 cores exchange their normalized attention outputs
with one AllToAll (bf16, 512KB) so that core c ends up with ALL 2048
attention features for its 128-position output stripe (batch c//4, stripe
c%4); it then applies the full wo to produce disjoint output rows. No
reduction collective is needed.
"""

import sys

sys.path.insert(0, "/opt/trn_rl_repo")

import numpy as np

import concourse.bass as bass
import concourse.bacc as bacc
import concourse.mybir as mybir
from concourse import tile
from concourse.bass_utils import run_bass_kernel_spmd

B, S, D = 2, 2048, 2048
N_HEADS, N_KV, HD = 32, 8, 64
NCORES = 8
QH = 4    # q-heads per core
FQ = QH * HD       # 256 q-feature cols per core
FKV = 2 * HD       # 128 (K then V) per core
SCALE = 1.0 / 8.0  # 1/sqrt(HD)

QTILE = 512
KTILE = 128
NSLAB = S // QTILE  # 4
ND = D // 128       # 16 contraction chunks

F32 = mybir.dt.float32
EXP = mybir.ActivationFunctionType.Exp
BF16 = mybir.dt.bfloat16
MMD = BF16
LE = mybir.AluOpType.is_ge


def _build_kernel(tc, io):
    nc = tc.nc
    xT, wq, wkv, wo = io["xT"], io["wq"], io["wkv"], io["wo"]
    cos2, sin2s, sel = io["cos2"], io["sin2s"], io["sel"]
    out_full = io["out"]
    single = bool(io.get("single"))

    # ---------------- pools ----------------
    const = tc.alloc_tile_pool(name="const", bufs=1)
    wpool = tc.alloc_tile_pool(name="wpool", bufs=1, side="right")
    kvp = tc.alloc_tile_pool(name="kvp", bufs=1)
    xpool = tc.alloc_tile_pool(name="xpool", bufs=2)
    qpool = tc.alloc_tile_pool(name="qpool", bufs=2)
    aop = tc.alloc_tile_pool(name="aop", bufs=2, side="right")
    rp = tc.alloc_tile_pool(name="rp", bufs=2)
    pexp = tc.alloc_tile_pool(name="pexp", bufs=4)
    evac = tc.alloc_tile_pool(name="evac", bufs=2)
    aogp = tc.alloc_tile_pool(name="aogp", bufs=2, side="right")
    dram = tc.alloc_tile_pool(name="dram", bufs=1, space="DRAM")

    psM = tc.alloc_tile_pool(name="psM", bufs=2, space="PSUM")
    psS = tc.alloc_tile_pool(name="psS", bufs=2, space="PSUM")
    psO = tc.alloc_tile_pool(name="psO", bufs=1, space="PSUM")

    # ------- constants + weights; DMA order tuned for fast start -------
    cos2_t = const.tile([128, S], MMD)
    nc.sync.dma_start(cos2_t[:], cos2[:])
    sin2s_t = const.tile([128, S], MMD)
    nc.sync.dma_start(sin2s_t[:], sin2s[:])
    ident = const.tile([128, 64], F32)
    nc.gpsimd.memset(ident[:], 0.0)
    for p in (0, 64):
        nc.gpsimd.affine_select(
            out=ident[p:p + 64, :], in_=ident[p:p + 64, :],
            compare_op=mybir.AluOpType.not_equal,
            fill=1.0, base=0, pattern=[[-1, 64]], channel_multiplier=1,
        )
    # x slab for (b=0, j=0) interleaved with the projection weights, spread
    # over both HW DMA queues so the first matmul group starts within ~5us
    xts00 = []
    WQ = {}
    WKV = {}
    for k in range(ND):
        q1, q2 = (nc.sync, nc.scalar) if k % 2 == 0 else (nc.scalar, nc.sync)
        xt = xpool.tile([128, QTILE], MMD, name="xt", tag=f"xt{k}")
        q1.dma_start(xt[:], xT[k * 128:(k + 1) * 128, 0:QTILE])
        xts00.append(xt)
        for t in range(2):
            w = wpool.tile([128, 128], MMD, name=f"wq{t}_{k}")
            q2.dma_start(w[:], wq[k * 128:(k + 1) * 128,
                                  t * 128:(t + 1) * 128])
            WQ[t, k] = w
        w = wpool.tile([128, 128], MMD, name=f"wkv{k}")
        q1.dma_start(w[:], wkv[k * 128:(k + 1) * 128, :])
        WKV[k] = w

    sel_t = const.tile([2 * QH, 4 * KTILE], MMD)
    nc.sync.dma_start(sel_t[:], sel[:])

    # full wo (loaded via the scalar DMA queue; scalar is idle early on)
    WO = {}
    for fc in range(ND):
        for dn in range(4):
            w = wpool.tile([128, QTILE], MMD, name=f"wo{fc}_{dn}")
            nc.scalar.dma_start(
                w[:], wo[fc * 128:(fc + 1) * 128,
                         dn * QTILE:(dn + 1) * QTILE])
            WO[fc, dn] = w

    # persistent K/V cache tiles
    KK = [kvp.tile([128, S], MMD, name=f"kk{b}") for b in range(B)]
    VA = {}
    for b in range(B):
        for i in range(S // KTILE):
            VA[b, i] = kvp.tile([128, HD + 1], MMD, name=f"va{b}_{i}")

    # A2A dram tiles (one pair per slab)
    a2a_in = [dram.tile([FQ * NCORES, KTILE], MMD, name=f"ain{j}")
              for j in range(NSLAB)]
    a2a_out = [dram.tile([FQ * NCORES, KTILE], MMD, name=f"aout{j}")
               for j in range(NSLAB)]

    AO = {}   # per (b, t) slab-local attention output, feature-major
    QT = {}

    def rope(dst, rows, qs, tab_qs):
        # dst[rows, qs] = dst*cos + swap32(dst)*sin  (feature-major RoPE);
        # qs indexes dst columns, tab_qs the (global-position) rope tables
        n = rows[1] - rows[0]
        qsw = rp.tile([128, QTILE], MMD, name="qsw", tag="qsw")
        for p in range(rows[0], rows[1], 64):
            q0 = p - rows[0]
            nc.sync.dma_start(qsw[q0:q0 + 32, :], dst[p + 32:p + 64, qs])
            nc.sync.dma_start(qsw[q0 + 32:q0 + 64, :], dst[p:p + 32, qs])
        t1 = rp.tile([128, QTILE], F32, name="t1", tag="t1")
        nc.vector.tensor_mul(t1[:n], dst[rows[0]:rows[1], qs],
                             cos2_t[rows[0]:rows[1], tab_qs])
        t2 = rp.tile([128, QTILE], F32, name="t2", tag="t2")
        nc.vector.tensor_mul(t2[:n], qsw[:n], sin2s_t[rows[0]:rows[1], tab_qs])
        nc.vector.tensor_add(dst[rows[0]:rows[1], qs], t1[:n], t2[:n])

    def prefetch_x(b, j):
        qs = slice(j * QTILE, (j + 1) * QTILE)
        xts = []
        for k in range(ND):
            xt = xpool.tile([128, QTILE], MMD, name="xt", tag=f"xt{k}")
            nc.sync.dma_start(
                xt[:], xT[b * D + k * 128:b * D + (k + 1) * 128, qs])
            xts.append(xt)
        return xts

    def make_proj_fillers(b, j, xts):
        # projection for (b, j), split into small PE chunks so it can be
        # woven into the preceding attention's exp-bound inner loop
        qs = slice(j * QTILE, (j + 1) * QTILE)
        ctx = {}
        fillers = []
        if xts is not None:
            ctx["x"] = xts
        else:
            def loadx():
                ctx["x"] = prefetch_x(b, j)
            fillers.append(loadx)
        for f in range(3):
            for sub in range(8):
                def mmchunk(f=f, sub=sub):
                    if sub == 0:
                        ctx[f] = psM.tile([128, QTILE], F32, name="psq",
                                          tag="mm")
                    ps = ctx[f]
                    for k in range(2 * sub, 2 * sub + 2):
                        w = WQ[f, k] if f < 2 else WKV[k]
                        nc.tensor.matmul(ps[:], w[:], ctx["x"][k][:],
                                         start=(k == 0), stop=(k == ND - 1))
                fillers.append(mmchunk)

            def evacf(f=f):
                ps = ctx[f]
                if f < 2:
                    qt = qpool.tile([128, QTILE], MMD, name="qt",
                                    tag=f"qt{b}_{f}")
                    QT[b, f] = qt
                    nc.vector.tensor_copy(qt[:], ps[:])
                    rope(qt, (0, 128), slice(0, QTILE), qs)
                else:
                    nc.vector.tensor_copy(KK[b][0:64, qs], ps[0:64, :])
                    rope(KK[b], (0, 64), qs, qs)
                    # duplicate roped K into rows 64:128 (row-tiled scores)
                    nc.sync.dma_start(KK[b][64:128, qs], KK[b][0:64, qs])
                    vv = rp.tile([128, QTILE], F32, name="vv", tag="vv")
                    nc.vector.tensor_copy(vv[64:128, :], ps[64:128, :])
                    ctx["vv"] = vv
            fillers.append(evacf)
        for c in range(4):
            def vtrans(c=c):
                i = 4 * j + c
                tp = psM.tile([128, QTILE], F32, name="tp", tag="mm")
                vv = ctx["vv"]
                nc.tensor.matmul(tp[:, 0:HD],
                                 vv[64:128, c * 128:(c + 1) * 128],
                                 ident[64:128, :], is_transpose=True,
                                 start=True, stop=True)
                va = VA[b, i]
                nc.vector.tensor_copy(va[:, 0:HD], tp[:, 0:HD])
                nc.vector.memset(va[:, HD:HD + 1], 1.0)
            fillers.append(vtrans)
        return fillers

    def proj(b, j, xts=None):
        for f in make_proj_fillers(b, j, xts):
            f()

    def attn(b, j, fillers=None):
        # fillers: closures emitting small independent PE chunks (wo/proj
        # work); paced evenly through the loop and placed before each attnV
        # so the tensor engine has work while it would otherwise stall on
        # the exp
        fillers = list(fillers or [])
        nkt = 4 * j + 4
        slots = 2 * nkt
        rate = len(fillers) / slots if slots else 0.0
        acc = [0.0]

        def pop_fillers():
            acc[0] += rate
            while fillers and acc[0] >= 1.0:
                fillers.pop(0)()
                acc[0] -= 1.0
        for t in range(2):
            oA = psO.tile([HD + 1, QTILE], F32, name="oA", tag="oA")
            oB = psO.tile([HD + 1, QTILE], F32, name="oB", tag="oB")
            sabs = {}

            def scores(i):
                r = i - 4 * j
                off = max(r, 0) * KTILE
                ks = slice(i * KTILE, (i + 1) * KTILE)
                sAB = psS.tile([128, 2 * QTILE], F32, name="sAB", tag="sAB")
                nc.tensor.matmul(sAB[:, off:QTILE], KK[b][0:64, ks],
                                 QT[b, t][0:64, off:], start=True, stop=True,
                                 tile_position=(0, 0))
                nc.tensor.matmul(sAB[:, QTILE + off:], KK[b][64:128, ks],
                                 QT[b, t][64:128, off:], start=True, stop=True,
                                 tile_position=(64, 0))
                sabs[i] = sAB

            scores(0)
            for i in range(nkt):
                r = i - 4 * j
                off = max(r, 0) * KTILE
                if i + 1 < nkt:
                    scores(i + 1)
                sAB = sabs.pop(i)
                pAB = pexp.tile([128, 2 * QTILE], MMD, name="pAB", tag="pAB")
                nc.scalar.activation(pAB[:, off:], sAB[:, off:], EXP,
                                     scale=SCALE)
                if r >= 0:
                    for h in (off, QTILE + off):
                        # zero the strictly-upper triangle (causal mask);
                        # gpsimd: keeps the DVE queue off this critical path
                        nc.gpsimd.affine_select(
                            out=pAB[:, h:h + KTILE], in_=pAB[:, h:h + KTILE],
                            compare_op=LE, fill=0.0, base=0,
                            pattern=[[1, KTILE]], channel_multiplier=-1)
                pop_fillers()
                nc.tensor.matmul(oA[:, off:], VA[b, i][:], pAB[:, off:QTILE],
                                 start=(i == 0), stop=(i == nkt - 1))
                nc.tensor.matmul(oB[:, off:], VA[b, i][:], pAB[:, QTILE + off:],
                                 start=(i == 0), stop=(i == nkt - 1))
            tA = evac.tile([HD + 1, QTILE], MMD, name="tA", tag="tA")
            tB = evac.tile([HD + 1, QTILE], MMD, name="tB", tag="tB")
            nc.vector.tensor_copy(tA[:], oA[:])
            nc.vector.tensor_copy(tB[:], oB[:])
            ao = AO[b, t]
            nc.sync.dma_start(ao[0:64, :], tA[0:64, :])
            nc.sync.dma_start(ao[64:128, :], tB[0:64, :])
            dn = AO["dn"]
            nc.sync.dma_start(dn[4 * b + t:4 * b + t + 1, :], tA[64:65, :])
            nc.sync.dma_start(dn[4 * b + 2 + t:4 * b + 3 + t, :],
                              tB[64:65, :])
        for f in fillers:
            f()

    def finish(j, AOj):
        # normalize, build the A2A input, kick the A2A
        dn = AOj["dn"]
        dnR = evac.tile([2 * QH, QTILE], F32, name="dnR", tag="dnR")
        nc.vector.reciprocal(dnR[:], dn[:])
        dnRb = evac.tile([2 * QH, QTILE], MMD, name="dnRb", tag="dnRb")
        nc.vector.tensor_copy(dnRb[:], dnR[:])
        for b in range(B):
            for t in range(2):
                bc = psM.tile([128, QTILE], F32, name="bc", tag="mm")
                nc.tensor.matmul(
                    bc[:], sel_t[:, (2 * b + t) * 128:(2 * b + t + 1) * 128],
                    dnRb[:], start=True, stop=True)
                nc.vector.tensor_mul(AOj[b, t][:], AOj[b, t][:], bc[:])
        for d in range(NCORES):
            bd, g = d // 4, d % 4
            for t in range(2):
                nc.sync.dma_start(
                    a2a_in[j][FQ * d + 128 * t:FQ * d + 128 * (t + 1), :],
                    AOj[bd, t][:, g * KTILE:(g + 1) * KTILE])
        if single:
            nc.sync.dma_start(a2a_out[j][:], a2a_in[j][:])
        else:
            nc.gpsimd.collective_compute(
                "AllToAll", mybir.AluOpType.bypass,
                replica_groups=[list(range(NCORES))],
                ins=[a2a_in[j][:]], outs=[a2a_out[j][:]],
            )

    def make_wo_fillers(j):
        # wo for slab j, split into one gather step plus 4x4 matmul chunks
        ctx = {}

        def gather():
            ctx["aogs"] = []
            for fc in range(ND):
                aog = aogp.tile([128, KTILE], MMD, name="aog", tag=f"aog{fc}")
                nc.sync.dma_start(aog[:],
                                  a2a_out[j][fc * 128:(fc + 1) * 128, :])
                ctx["aogs"].append(aog)

        fillers = [gather]
        for dn in range(4):
            for sub in range(8):
                def chunk(dn=dn, sub=sub):
                    if sub == 0:
                        ctx[dn] = psM.tile([128, QTILE], F32, name="psW",
                                           tag="mm")
                    ps = ctx[dn]
                    for fc in range(2 * sub, 2 * sub + 2):
                        nc.tensor.matmul(ps[:], ctx["aogs"][fc][:],
                                         WO[fc, dn][:],
                                         start=(fc == 0), stop=(fc == ND - 1))
                    if sub == 7:
                        og = evac.tile([128, QTILE], F32, name="og", tag="og")
                        nc.vector.tensor_copy(og[:], ps[:])
                        nc.gpsimd.dma_start(
                            out_full[j * 128:(j + 1) * 128,
                                     dn * QTILE:(dn + 1) * QTILE], og[:])
                fillers.append(chunk)
        return fillers

    def wo_slab(j):
        for f in make_wo_fillers(j):
            f()

    def interleave(*lists):
        out = []
        idx = [0] * len(lists)
        while any(i < len(l) for i, l in zip(idx, lists)):
            for n, l in enumerate(lists):
                if idx[n] < len(l):
                    out.append(l[idx[n]])
                    idx[n] += 1
        return out

    proj(0, 0, xts00)
    pend = None
    for j in range(NSLAB):
        AO.clear()
        AO["dn"] = aop.tile([2 * QH, QTILE], MMD, name="dn", tag="dn")
        for b in range(B):
            AO[b, 0] = aop.tile([128, QTILE], MMD, name=f"ao{b}0",
                                tag=f"ao{b}0")
            AO[b, 1] = aop.tile([128, QTILE], MMD, name=f"ao{b}1",
                                tag=f"ao{b}1")
        if pend is not None:
            finish(*pend)
        # weave proj(1, j) into attn(0, j)'s exp-bound loop
        attn(0, j, fillers=make_proj_fillers(1, j, None if j == 0 else nxt1))
        # weave the previous slab's wo and the next slab's proj(0) into
        # attn(1, j)
        f_wo = make_wo_fillers(j - 1) if j > 0 else []
        if j + 1 < NSLAB:
            nxt0 = prefetch_x(0, j + 1)
            f_pj = make_proj_fillers(0, j + 1, nxt0)
        else:
            f_pj = []
        attn(1, j, fillers=f_pj + f_wo)
        nxt1 = prefetch_x(1, j + 1) if j + 1 < NSLAB else None
        pend = (j, dict(AO))
    finish(*pend)
    wo_slab(NSLAB - 1)

    for p in (psO, psS, psM, dram, aogp, evac, pexp, rp, aop, qpool, xpool,
              kvp, wpool, const):
        p.release()


def _build(single=False):
    nc = bacc.Bacc("TRN2", target_bir_lowering=False, debug=False,
                   num_devices=1 if single else NCORES)
    io = {
        "xT": nc.dram_tensor("xT", [B * D, S], BF16, kind="ExternalInput").ap(),
        "wq": nc.dram_tensor("wq", [D, FQ], BF16, kind="ExternalInput").ap(),
        "wkv": nc.dram_tensor("wkv", [D, FKV], BF16, kind="ExternalInput").ap(),
        "wo": nc.dram_tensor("wo", [D, D], BF16, kind="ExternalInput").ap(),
        "cos2": nc.dram_tensor("cos2", [128, S], BF16, kind="ExternalInput").ap(),
        "sin2s": nc.dram_tensor("sin2s", [128, S], BF16, kind="ExternalInput").ap(),
        "sel": nc.dram_tensor("sel", [2 * QH, 4 * KTILE], BF16,
                              kind="ExternalInput").ap(),
        "out": nc.dram_tensor("out", [NSLAB * 128, D], F32,
                              kind="ExternalOutput").ap(),
    }
    io["single"] = single
    with tile.TileContext(nc) as tc:
        _build_kernel(tc, io)
    nc.compile()
    return nc


_CACHE = {}


def _get_program():
    if "nc" not in _CACHE:
        _CACHE["nc"] = _build()
    return _CACHE["nc"]


def _host_inputs(x, wq, wk, wv, wo):
    x = np.ascontiguousarray(x, np.float32)
    inv = 1.0 / (10000.0 ** (np.arange(0, HD, 2, dtype=np.float64) / HD))
    pos = np.arange(S, dtype=np.float64)
    freqs = np.outer(pos, inv)                      # [S, 32]
    emb = np.concatenate([freqs, freqs], axis=1)    # [S, 64]
    cos = np.cos(emb).T.astype(np.float32)          # [64, S]
    sin = np.sin(emb).T.astype(np.float32)
    cos2 = np.concatenate([cos, cos], axis=0)       # [128, S]
    sin2s = np.concatenate([-sin[:32], sin[32:], -sin[:32], sin[32:]], axis=0)

    # denominator broadcast selector: for (b, t) block, AO[b,t] rows 0:64
    # <- dn row 4b+t, rows 64:128 <- dn row 4b+2+t
    sel = np.zeros((2 * QH, 4 * KTILE), np.float32)
    for b in range(2):
        for t in range(2):
            blk = (2 * b + t) * 128
            sel[4 * b + t, blk:blk + 64] = 1.0
            sel[4 * b + 2 + t, blk + 64:blk + 128] = 1.0

    import ml_dtypes
    bf16 = ml_dtypes.bfloat16
    cos2 = cos2.astype(bf16)
    sin2s = sin2s.astype(bf16)
    sel = sel.astype(bf16)
    xT = np.ascontiguousarray(
        np.concatenate([x[0].T, x[1].T], axis=0).astype(bf16))  # [2D, S]

    # wo rows ordered to match the gathered A2A feature order:
    # src core cc contributes heads (4cc+t, 4cc+t+2) for t in (0, 1)
    wrows = []
    for cc in range(NCORES):
        for t in range(2):
            for h in (4 * cc + t, 4 * cc + t + 2):
                wrows.append(wo[h * HD:(h + 1) * HD, :])
    wo_p = np.ascontiguousarray(np.concatenate(wrows, axis=0).astype(bf16))

    in_maps = []
    for c in range(NCORES):
        qcols = []
        for t in range(2):
            for h in (4 * c + t, 4 * c + t + 2):
                qcols.append(wq[:, h * HD:(h + 1) * HD])
        wq_p = np.ascontiguousarray(np.concatenate(qcols, axis=1).astype(bf16))
        wkv_p = np.ascontiguousarray(np.concatenate(
            [wk[:, c * HD:(c + 1) * HD], wv[:, c * HD:(c + 1) * HD]],
            axis=1).astype(bf16))
        in_maps.append({
            "xT": xT, "wq": wq_p, "wkv": wkv_p, "wo": wo_p,
            "cos2": cos2, "sin2s": sin2s, "sel": sel,
        })
    return in_maps


def run(x, wq, wk, wv, wo, trace=False, **trace_kwargs):
    nc = _get_program()
    in_maps = _host_inputs(x, wq, wk, wv, wo)
    res = run_bass_kernel_spmd(nc, in_maps, list(range(NCORES)),
                               trace=trace, **trace_kwargs)
    out = np.empty((B, S, D), np.float32)
    for c in range(NCORES):
        bo, g = c // 4, c % 4
        shard = res.results[c]["out"]  # [512, D]
        for j in range(NSLAB):
            out[bo, j * QTILE + g * 128:j * QTILE + (g + 1) * 128, :] = \
                shard[j * 128:(j + 1) * 128, :]
    return out, res


def kernel(x, wq, wk, wv, wo):
    out, _ = run(x, wq, wk, wv, wo)
    return out.astype(np.float32)
